# revision 1
# baseline (speedup 1.0000x reference)
"""Trainium2 Bass kernel for 3-layer GAT + global max pool + MLP (nn_ATTGCN).

Distribution: 8 cores; core c owns dst-nodes [6250c, 6250(c+1)) and graphs
[32c, 32(c+1)). Per layer: local dense (h@W + attention logit columns) ->
AllGather of augmented rows [h+bc | e_s | 1] -> edge phase (indirect-gather
rows by src, one-hot matmul scatter-accumulate per <=128-node group with the
softmax denominator as an extra accumulated column). Pooling + MLP local.

Self-contained: hardcodes N=50000, E=640000, G=256, F=H=128, L=3, C=10.
"""
import numpy as np

N, E, F, G, C, L = 50000, 640000, 128, 256, 10, 3
NCORES = 8
NPC = N // NCORES            # 6250
NT = (NPC + 127) // 128      # 49
NPAD = NT * 128              # 6272
CHUNK = 128
GCHUNKS = 8
GEDGES = CHUNK * GCHUNKS     # 1024
DUMP = 128
ROWW = 130                   # h(128) | e_s | 1
NEG_SLOPE = 0.2
GB = 8                       # groups per gather batch


def _build_tables(edge_index):
    src_all = np.concatenate([edge_index[0], np.arange(N, dtype=np.int64)]).astype(np.int64)
    dst_all = np.concatenate([edge_index[1], np.arange(N, dtype=np.int64)]).astype(np.int64)
    order = np.argsort(dst_all, kind="stable")
    src_all, dst_all = src_all[order], dst_all[order]
    owner = src_all // NPC
    src_aug_row = owner * NPAD + (src_all - owner * NPC)

    cores = []
    max_ng = 0
    bounds = np.searchsorted(dst_all, np.arange(0, N + 1, NPC))
    for c in range(NCORES):
        lo, hi = bounds[c], bounds[c + 1]
        s_aug = src_aug_row[lo:hi]
        d_loc = (dst_all[lo:hi] - c * NPC).astype(np.int64)
        deg = np.bincount(d_loc, minlength=NPC)
        groups = []
        nb, ne, eb, ecnt = 0, 0, 0, 0
        for n in range(NPC):
            d = int(deg[n])
            if ecnt + d > GEDGES or ne == 128:
                groups.append((nb, ne, eb, eb + ecnt))
                nb, ne, eb, ecnt = n, 0, eb + ecnt, 0
            ne += 1
            ecnt += d
        groups.append((nb, ne, eb, eb + ecnt))
        max_ng = max(max_ng, len(groups))
        cores.append((s_aug, d_loc, groups))

    NG = max_ng
    NCH = NG * GCHUNKS
    tabs = []
    for c in range(NCORES):
        s_aug, d_loc, groups = cores[c]
        src_off = np.zeros((CHUNK, NCH), np.int32)
        dst_rel = np.full((CHUNK, NCH), -1.0, np.float32)
        edl_off = np.zeros((CHUNK, NCH), np.int32)
        hn_off = np.zeros((CHUNK, NG), np.int32)
        hn_off[:, :] = NPAD + np.arange(CHUNK)[:, None]
        for g, (nb, nn, el, eh) in enumerate(groups):
            ne = eh - el
            sbuf = np.zeros(GEDGES, np.int64)
            rbuf = np.full(GEDGES, -1.0, np.float32)
            ebuf = np.zeros(GEDGES, np.int64)
            sbuf[:ne] = s_aug[el:eh]
            rbuf[:ne] = (d_loc[el:eh] - nb).astype(np.float32)
            ebuf[:ne] = (d_loc[el:eh] % 128) * NT + d_loc[el:eh] // 128
            for k in range(GCHUNKS):
                col = g * GCHUNKS + k
                src_off[:, col] = sbuf[k * 128:(k + 1) * 128]
                dst_rel[:, col] = rbuf[k * 128:(k + 1) * 128]
                edl_off[:, col] = ebuf[k * 128:(k + 1) * 128]
            hn_off[:nn, g] = nb + np.arange(nn)
        tabs.append(dict(src_off=src_off, dst_rel=dst_rel,
                         edl_off=edl_off, hn_off=hn_off))

    pool_off = []
    gb = np.ceil(np.arange(G + 1) * N / G).astype(np.int64)
    for c in range(NCORES):
        po = np.zeros((CHUNK, 64), np.int32)
        for gi in range(32):
            g = 32 * c + gi
            lo, hi = gb[g] - c * NPC, gb[g + 1] - c * NPC
            cnt = hi - lo
            idx = np.full(256, lo, np.int64)
            idx[:cnt] = np.arange(lo, hi)
            po[:, 2 * gi] = idx[:128]
            po[:, 2 * gi + 1] = idx[128:]
        pool_off.append(po)
    return NG, tabs, pool_off


def _build_bass(NG, deltas=(0.0, 0.0, 0.0), phase="full"):
    import concourse.bass as bass
    import concourse.bacc as bacc
    import concourse.mybir as mybir
    import concourse.tile as tile

    f32 = mybir.dt.float32
    i32 = mybir.dt.int32
    AF = mybir.ActivationFunctionType
    OP = mybir.AluOpType
    AX = mybir.AxisListType
    IOff = bass.IndirectOffsetOnAxis
    NCH = NG * GCHUNKS

    nc = bacc.Bacc("TRN2", target_bir_lowering=False, debug=False,
                   num_devices=NCORES)

    # ---- external I/O (per core) ----
    xT_d = nc.dram_tensor("xT", [128, NPAD], f32, kind="ExternalInput").ap()
    Wc_d = nc.dram_tensor("Wc", [128, L * 128], f32, kind="ExternalInput").ap()
    asd_d = nc.dram_tensor("asd", [128, L * 2], f32, kind="ExternalInput").ap()
    bc_d = nc.dram_tensor("bc", [128, L], f32, kind="ExternalInput").ap()
    W1_d = nc.dram_tensor("W1", [128, 128], f32, kind="ExternalInput").ap()
    b1_d = nc.dram_tensor("b1", [128, 1], f32, kind="ExternalInput").ap()
    W2_d = nc.dram_tensor("W2", [128, C], f32, kind="ExternalInput").ap()
    b2_d = nc.dram_tensor("b2", [C, 1], f32, kind="ExternalInput").ap()
    iota_d = nc.dram_tensor("iota", [128, 128], f32, kind="ExternalInput").ap()
    ident_d = nc.dram_tensor("ident", [128, 128], f32, kind="ExternalInput").ap()
    srco_d = nc.dram_tensor("src_off", [128, NCH], i32, kind="ExternalInput").ap()
    dstr_d = nc.dram_tensor("dst_rel", [128, NCH], f32, kind="ExternalInput").ap()
    edlo_d = nc.dram_tensor("edl_off", [128, NCH], i32, kind="ExternalInput").ap()
    hno_d = nc.dram_tensor("hn_off", [128, NG], i32, kind="ExternalInput").ap()
    poolo_d = nc.dram_tensor("pool_off", [128, 64], i32, kind="ExternalInput").ap()
    y_d = nc.dram_tensor("y", [32, C], f32, kind="ExternalOutput").ap()
    if phase == "d0":
        dbg_d = nc.dram_tensor("dbg", [NPAD, ROWW], f32, kind="ExternalOutput").ap()
    elif phase == "ag0":
        dbg_d = nc.dram_tensor("dbg", [NPAD, ROWW], f32, kind="ExternalOutput").ap()
    elif phase in ("e0", "e1", "e2"):
        dbg_d = nc.dram_tensor("dbg", [NPAD + DUMP, 128], f32, kind="ExternalOutput").ap()
    elif phase == "g0":
        dbg_d = nc.dram_tensor("dbg", [128, 64 * ROWW + 64], f32, kind="ExternalOutput").ap()
    else:
        dbg_d = None

    with tile.TileContext(nc) as tc:
        import contextlib
        with contextlib.ExitStack() as ctx:
            cpool = ctx.enter_context(tc.tile_pool(name="consts", bufs=1))
            dram = ctx.enter_context(tc.tile_pool(name="dram", bufs=1, space="DRAM"))

            def const(cname, shape, dt, src):
                t = cpool.tile(shape, dt, name=cname, tag=cname)
                nc.sync.dma_start(t[:], src)
                return t

            xT = const("c_xT", [128, NPAD], f32, xT_d[:])
            Wc = const("c_Wc", [128, L * 128], f32, Wc_d[:])
            asd = const("c_asd", [128, L * 2], f32, asd_d[:])
            bcc = const("c_bcc", [128, L], f32, bc_d[:])
            W1 = const("c_W1", [128, 128], f32, W1_d[:])
            b1 = const("c_b1", [128, 1], f32, b1_d[:])
            W2 = const("c_W2", [128, C], f32, W2_d[:])
            b2 = const("c_b2", [C, 1], f32, b2_d[:])
            iota = const("c_iota", [128, 128], f32, iota_d[:])
            ident = const("c_ident", [128, 128], f32, ident_d[:])
            srco = const("c_srco", [128, NCH], i32, srco_d[:])
            dstr = const("c_dstr", [128, NCH], f32, dstr_d[:])
            edlo = const("c_edlo", [128, NCH], i32, edlo_d[:])
            hno = const("c_hno", [128, NG], i32, hno_d[:])
            poolo = const("c_poolo", [128, 64], i32, poolo_d[:])

            ag_in = [dram.tile([NPAD, ROWW], f32, name=f"ag_in{l}")
                     for l in range(L)]
            h_aug = [dram.tile([NCORES * NPAD, ROWW], f32, addr_space="Shared",
                               name=f"h_aug{l}") for l in range(L)]
            edl = [dram.tile([NPAD, 1], f32, name=f"edl{l}") for l in range(L)]
            h_next = [dram.tile([NPAD + DUMP, 128], f32, name=f"h_next{l}")
                      for l in range(L)]

            dpool = ctx.enter_context(tc.tile_pool(name="dense", bufs=3))
            edpool = ctx.enter_context(tc.tile_pool(name="edcol", bufs=1))
            ps_h = ctx.enter_context(tc.tile_pool(name="psh", bufs=2, space="PSUM"))
            ps_t = ctx.enter_context(tc.tile_pool(name="pst", bufs=2, space="PSUM"))
            ps_e = ctx.enter_context(tc.tile_pool(name="pse", bufs=2, space="PSUM"))

            # ---------------- dense phase ----------------
            def dense(l):
                ed_stage = edpool.tile([128, NT], f32, tag="edstage")
                for t in range(NT):
                    if l == 0:
                        rhsT = xT[:, t * 128:(t + 1) * 128]
                    else:
                        rows_in = dpool.tile([128, 128], f32, tag="rows_in")
                        nc.sync.dma_start(
                            rows_in[:], h_next[l - 1][:].rearrange(
                                "(t p) f -> t p f", p=128)[t])
                        pT = ps_t.tile([128, 128], f32, tag='pst')
                        nc.tensor.transpose(pT[:], rows_in[:], ident[:])
                        hTin = dpool.tile([128, 128], f32, tag="hTin")
                        nc.scalar.activation(hTin[:], pT[:], AF.Identity)
                        rhsT = hTin[:]
                    ph = ps_h.tile([128, 128], f32, tag='psh')
                    nc.tensor.matmul(ph[:], lhsT=Wc[:, l * 128:(l + 1) * 128],
                                     rhs=rhsT, start=True, stop=True)
                    hTb = dpool.tile([128, 128], f32, tag="hTb")
                    nc.scalar.activation(hTb[:], ph[:], AF.Identity,
                                         bias=bcc[:, l:l + 1])
                    pe = ps_e.tile([128, 2], f32, tag='pse')
                    nc.tensor.matmul(pe[:], lhsT=hTb[:],
                                     rhs=asd[:, 2 * l:2 * l + 2],
                                     start=True, stop=True)
                    pr = ps_t.tile([128, 128], f32, tag='pst')
                    nc.tensor.transpose(pr[:], hTb[:], ident[:])
                    rows = dpool.tile([128, ROWW], f32, tag="rows")
                    nc.vector.tensor_copy(rows[:, 0:128], pr[:])
                    nc.vector.tensor_copy(rows[:, 128:129], pe[:, 0:1])
                    nc.vector.memset(rows[:, 129:130], 1.0)
                    nc.scalar.activation(ed_stage[:, t:t + 1], pe[:, 1:2],
                                         AF.Identity, bias=-float(deltas[l]))
                    nc.sync.dma_start(
                        ag_in[l][:].rearrange("(t p) f -> t p f", p=128)[t],
                        rows[:])
                nc.sync.dma_start(
                    edl[l][:].rearrange("(p t) o -> p (t o)", p=128),
                    ed_stage[:])

            # ---------------- edge phase ----------------
            gpool = ctx.enter_context(tc.tile_pool(name="gath", bufs=2))
            epool = ctx.enter_context(tc.tile_pool(name="ede", bufs=2))
            wpool = ctx.enter_context(tc.tile_pool(name="wchain", bufs=2))
            apool = ctx.enter_context(tc.tile_pool(name="amat", bufs=4))
            spool = ctx.enter_context(tc.tile_pool(name="small", bufs=8))
            hpool = ctx.enter_context(tc.tile_pool(name="hnb", bufs=2))
            ps_a = ctx.enter_context(tc.tile_pool(name="psagg", bufs=2, space="PSUM"))

            def edge(l):
                nb = (NG + GB - 1) // GB
                for b in range(nb):
                    g0 = b * GB
                    gn = min(GB, NG - g0)
                    cols = slice(g0 * GCHUNKS, (g0 + gn) * GCHUNKS)
                    ncol = gn * GCHUNKS
                    gts = gpool.tile([128, GB * GCHUNKS, ROWW], f32, tag="gts")
                    ede = epool.tile([128, GB * GCHUNKS], f32, tag="ede")
                    for cc in range(ncol):
                        so1 = epool.tile([128, 1], i32, tag=f"so1_{cc % 4}")
                        nc.vector.tensor_copy(
                            so1[:], srco[:, g0 * GCHUNKS + cc:g0 * GCHUNKS + cc + 1])
                        nc.gpsimd.indirect_dma_start(
                            out=gts[:, cc, :], out_offset=None,
                            in_=h_aug[l][:],
                            in_offset=IOff(ap=so1[:], axis=0))
                        eo1 = epool.tile([128, 1], i32, tag=f"eo1_{cc % 4}")
                        nc.vector.tensor_copy(
                            eo1[:], edlo[:, g0 * GCHUNKS + cc:g0 * GCHUNKS + cc + 1])
                        nc.gpsimd.indirect_dma_start(
                            out=ede[:, cc:cc + 1], out_offset=None,
                            in_=edl[l][:],
                            in_offset=IOff(ap=eo1[:], axis=0))
                    s = wpool.tile([128, GB * GCHUNKS], f32, tag="s")
                    nc.vector.tensor_tensor(
                        out=s[:, 0:ncol],
                        in0=gts[:, 0:ncol, 128],
                        in1=ede[:, 0:ncol], op=OP.add)
                    t2 = wpool.tile([128, GB * GCHUNKS], f32, tag="t2")
                    nc.vector.tensor_scalar(
                        out=t2[:, 0:ncol], in0=s[:, 0:ncol],
                        scalar1=NEG_SLOPE, scalar2=None, op0=OP.mult)
                    lk = wpool.tile([128, GB * GCHUNKS], f32, tag="lk")
                    nc.vector.tensor_tensor(
                        out=lk[:, 0:ncol], in0=s[:, 0:ncol],
                        in1=t2[:, 0:ncol], op=OP.max)
                    w = wpool.tile([128, GB * GCHUNKS], f32, tag="w")
                    nc.scalar.activation(w[:, 0:ncol], lk[:, 0:ncol], AF.Exp)
                    hnb = hpool.tile([128, GB, 128], f32, tag="hnb")
                    for gi in range(gn):
                        pa = ps_a.tile([128, ROWW], f32, tag='psa')
                        for k in range(GCHUNKS):
                            j = gi * GCHUNKS + k
                            col = (g0 + gi) * GCHUNKS + k
                            A = apool.tile([128, 128], f32, tag="A")
                            nc.vector.tensor_scalar(
                                out=A[:], in0=iota[:],
                                scalar1=dstr[:, col:col + 1],
                                scalar2=w[:, j:j + 1],
                                op0=OP.is_equal, op1=OP.mult)
                            nc.tensor.matmul(pa[:], lhsT=A[:],
                                             rhs=gts[:, j, :],
                                             start=(k == 0), stop=(k == GCHUNKS - 1))
                        zb = spool.tile([128, 1], f32, tag="zb")
                        nc.vector.tensor_scalar(
                            out=zb[:], in0=pa[:, 129:130],
                            scalar1=1e-30, scalar2=None, op0=OP.add)
                        rz = spool.tile([128, 1], f32, tag="rz")
                        nc.vector.reciprocal(rz[:], zb[:])
                        nc.scalar.activation(hnb[:, gi, :], pa[:, 0:128],
                                             AF.Relu, scale=rz[:])
                    for gg in range(gn):
                        ho1 = epool.tile([128, 1], i32, tag=f"ho1_{gg % 2}")
                        nc.vector.tensor_copy(ho1[:], hno[:, g0 + gg:g0 + gg + 1])
                        nc.gpsimd.indirect_dma_start(
                            out=h_next[l][:],
                            out_offset=IOff(ap=ho1[:], axis=0),
                            in_=hnb[:, gg, :], in_offset=None)

            # ---------------- run the layers ----------------
            rg = [list(range(NCORES))]

            def finish_dbg(src_ap):
                tpool = tc.tile_pool(name="dbgp", bufs=2)
                with tpool as tp:
                    nrow, ncolw = src_ap.shape
                    for t in range(nrow // 128):
                        db = tp.tile([128, ncolw], f32, tag="dbgt")
                        view = src_ap.rearrange("(t p) f -> t p f", p=128)
                        nc.sync.dma_start(db[:], view[t])
                        nc.sync.dma_start(
                            dbg_d.rearrange("(t p) f -> t p f", p=128)[t], db[:])
                    yz = tp.tile([32, C], f32, tag="yz")
                    nc.vector.memset(yz[:], 0.0)
                    nc.sync.dma_start(y_d[:], yz[:])

            if phase == "g0":
                dense(0)
                nc.gpsimd.collective_compute(
                    "AllGather", mybir.AluOpType.bypass,
                    ins=[ag_in[0].opt()], outs=[h_aug[0].opt()],
                    replica_groups=rg)
                gpool0 = ctx.enter_context(tc.tile_pool(name="g0p", bufs=1))
                so0 = gpool0.tile([128, min(64, NCH)], i32)
                nc.vector.tensor_copy(so0[:], srco[:, 0:min(64, NCH)])
                eo0 = gpool0.tile([128, min(64, NCH)], i32)
                nc.vector.tensor_copy(eo0[:], edlo[:, 0:min(64, NCH)])
                gts0 = gpool0.tile([128, min(64, NCH), ROWW], f32)
                nc.gpsimd.indirect_dma_start(
                    out=gts0[:], out_offset=None, in_=h_aug[0][:],
                    in_offset=IOff(ap=so0[:], axis=0))
                ede0 = gpool0.tile([128, min(64, NCH)], f32)
                nc.gpsimd.indirect_dma_start(
                    out=ede0[:], out_offset=None, in_=edl[0][:],
                    in_offset=IOff(ap=eo0[:], axis=0))
                nc.sync.dma_start(dbg_d[:, 0:min(64, NCH) * ROWW],
                                  gts0[:].rearrange("p j f -> p (j f)"))
                nc.sync.dma_start(dbg_d[:, 64 * ROWW:64 * ROWW + min(64, NCH)], ede0[:])
                yz0 = gpool0.tile([32, C], f32)
                nc.vector.memset(yz0[:], 0.0)
                nc.sync.dma_start(y_d[:], yz0[:])
            elif phase in ("d0", "ag0", "e0", "e1", "e2"):
                nlay = int(phase[1]) if phase[0] == "e" else 0
                dense(0)
                if phase == "d0":
                    finish_dbg(ag_in[0][:])
                else:
                    nc.gpsimd.collective_compute(
                        "AllGather", mybir.AluOpType.bypass,
                        ins=[ag_in[0].opt()], outs=[h_aug[0].opt()],
                        replica_groups=rg)
                    if phase == "ag0":
                        finish_dbg(h_aug[0][:].rearrange(
                            "(c n) f -> c n f", c=NCORES)[0])
                    else:
                        edge(0)
                        for ll in range(1, nlay + 1):
                            dense(ll)
                            nc.gpsimd.collective_compute(
                                "AllGather", mybir.AluOpType.bypass,
                                ins=[ag_in[ll].opt()], outs=[h_aug[ll].opt()],
                                replica_groups=rg)
                            edge(ll)
                        finish_dbg(h_next[nlay][:])
            else:
              for l in range(L):
                  dense(l)
                  nc.gpsimd.collective_compute(
                      "AllGather", mybir.AluOpType.bypass,
                      ins=[ag_in[l].opt()], outs=[h_aug[l].opt()],
                      replica_groups=rg)
                  edge(l)

            if phase == "full":
              # ---------------- pooling + MLP ----------------
              pgpool = ctx.enter_context(tc.tile_pool(name="poolg", bufs=1))
              pt = pgpool.tile([128, 64, 128], f32)
              for cc in range(64):
                  po1 = pgpool.tile([128, 1], i32, tag=f"po1_{cc % 2}")
                  nc.vector.tensor_copy(po1[:], poolo[:, cc:cc + 1])
                  nc.gpsimd.indirect_dma_start(
                      out=pt[:, cc, :], out_offset=None, in_=h_next[L - 1][:],
                      in_offset=IOff(ap=po1[:], axis=0))
              gmax = pgpool.tile([128, 32], f32)
              for gi in range(32):
                  r01 = []
                  for half in range(2):
                      pr = ps_t.tile([128, 128], f32, tag='pst')
                      nc.tensor.transpose(pr[:], pt[:, 2 * gi + half, :], ident[:])
                      sb = dpool.tile([128, 128], f32, tag="pooltr")
                      nc.scalar.activation(sb[:], pr[:], AF.Identity)
                      rm = spool.tile([128, 1], f32, tag=f"rm{half}")
                      nc.vector.tensor_reduce(rm[:], sb[:], axis=AX.X, op=OP.max)
                      r01.append(rm)
                  nc.vector.tensor_tensor(out=gmax[:, gi:gi + 1], in0=r01[0][:],
                                          in1=r01[1][:], op=OP.max)
              pg1 = ps_h.tile([128, 32], f32, tag='psh')
              nc.tensor.matmul(pg1[:], lhsT=W1[:], rhs=gmax[:], start=True, stop=True)
              g1 = pgpool.tile([128, 32], f32)
              nc.scalar.activation(g1[:], pg1[:], AF.Relu, bias=b1[:])
              pl2 = ps_e.tile([C, 32], f32, tag='pse')
              nc.tensor.matmul(pl2[:], lhsT=W2[:], rhs=g1[:], start=True, stop=True)
              lgT = pgpool.tile([C, 32], f32)
              nc.scalar.activation(lgT[:], pl2[:], AF.Identity, bias=b2[:])
              plg = ps_t.tile([32, C], f32, tag='pst')
              nc.tensor.transpose(plg[:], lgT[:], ident[:C, :C])
              lg = pgpool.tile([32, C], f32)
              nc.vector.tensor_copy(lg[:], plg[:])
              mx = pgpool.tile([32, 1], f32)
              nc.vector.tensor_reduce(mx[:], lg[:], axis=AX.X, op=OP.max)
              tl = pgpool.tile([32, C], f32)
              nc.vector.tensor_scalar(out=tl[:], in0=lg[:], scalar1=mx[:],
                                      scalar2=None, op0=OP.subtract)
              ex = pgpool.tile([32, C], f32)
              nc.scalar.activation(ex[:], tl[:], AF.Exp)
              sm = pgpool.tile([32, 1], f32)
              nc.vector.tensor_reduce(sm[:], ex[:], axis=AX.X, op=OP.add)
              ln = pgpool.tile([32, 1], f32)
              nc.scalar.activation(ln[:], sm[:], AF.Ln)
              ysb = pgpool.tile([32, C], f32)
              nc.vector.tensor_scalar(out=ysb[:], in0=tl[:], scalar1=ln[:],
                                      scalar2=None, op0=OP.subtract)
              nc.sync.dma_start(y_d[:], ysb[:])

    nc.compile()
    return nc


_CACHE = {}


def _get_program(NG, deltas):
    key = (NG, deltas)
    if key not in _CACHE:
        _CACHE[key] = _build_bass(NG, deltas)
    return _CACHE[key]


def run(inputs, trace=False, tmpdir=None):
    from concourse import bass_utils
    x = np.asarray(inputs["x"], np.float32)
    edge_index = np.asarray(inputs["edge_index"], np.int64)
    Wc = np.asarray(inputs["Wc"], np.float32)
    a_src = np.asarray(inputs["a_src"], np.float32)
    a_dst = np.asarray(inputs["a_dst"], np.float32)
    bc = np.asarray(inputs["bc"], np.float32)
    W1 = np.asarray(inputs["W1"], np.float32)
    b1 = np.asarray(inputs["b1"], np.float32)
    W2 = np.asarray(inputs["W2"], np.float32)
    b2 = np.asarray(inputs["b2"], np.float32)

    NG, tabs, pool_off = _build_tables(edge_index)
    deltas = tuple(float(bc[l] @ a_src[l] + bc[l] @ a_dst[l]) for l in range(L))
    nc = _get_program(NG, deltas)

    Wc_in = np.concatenate([Wc[l] for l in range(L)], axis=1)
    asd_in = np.concatenate(
        [np.stack([a_src[l], a_dst[l]], axis=-1) for l in range(L)], axis=1)
    bc_in = np.ascontiguousarray(bc.T)
    iota = np.broadcast_to(np.arange(128, dtype=np.float32), (128, 128)).copy()
    ident = np.eye(128, dtype=np.float32)

    in_maps = []
    for c in range(NCORES):
        xTc = np.zeros((128, NPAD), np.float32)
        xTc[:, :NPC] = x[c * NPC:(c + 1) * NPC].T
        in_maps.append(dict(
            xT=xTc, Wc=Wc_in, asd=asd_in, bc=bc_in, W1=W1,
            b1=b1.reshape(128, 1), W2=W2, b2=b2.reshape(C, 1),
            iota=iota, ident=ident,
            src_off=tabs[c]["src_off"], dst_rel=tabs[c]["dst_rel"],
            edl_off=tabs[c]["edl_off"], hn_off=tabs[c]["hn_off"],
            pool_off=pool_off[c],
        ))
    res = bass_utils.run_bass_kernel_spmd(
        nc, in_maps, core_ids=list(range(NCORES)), trace=trace,
        tmpdir=tmpdir)
    out = np.concatenate([res.results[c]["y"] for c in range(NCORES)], axis=0)
    return out, res


def kernel(**inputs) -> np.ndarray:
    out, _ = run(inputs, trace=False)
    return out.astype(np.float32)

